# revision 20
# baseline (speedup 1.0000x reference)
"""Binary segmentation loss (dice + boundary + focal) on 8 Trainium2 cores.

Data parallel: image i -> core i. Each core computes six partial sums over
its image; the host combines them into the 4 scalar outputs.

v2 design notes (vs the v1 scan kernel):
- Single ACT function table (natural_log_exp_and_others): sigmoid is
  computed as p = 1/(1+exp(-x)) and sqrt(d2) as exp(0.5*ln(d2)), so the
  per-iteration LoadActFuncSet swaps (4 x 1283ns) disappear.
- The boundary term never transposes d2 back: it multiplies
  exp(0.5*ln(d2)) by a PE-transposed copy of p and accumulates in the
  x-major layout (sums are layout-invariant).
- Elementwise work is rebalanced across DVE (tensor_scalar/tensor_tensor
  fast paths), Pool (flat 427ns, incl. accumulating scalar_tensor_tensor)
  and ACT (4 ops), with PE doing all transposes.
- The EDT y-window is +-2 (WIN=2). Exactness certificate: if the windowed
  d2 is <= (WIN+1)^2 = 9 everywhere, any |dy|>WIN candidate (>= 9) cannot
  beat it, so the windowed result equals the exact EDT. A host-side guard
  verifies this and falls back to an exact numpy EDT per failing image
  (never fires for ~50% random masks, P ~ 1e-2 per full test run).

Stats columns (per partition, host sums over partitions):
  0: 2*sum(p*t)   1: sum(p+t)   2: sum(W^2*ln(pt))   3: sum(t*W^2*ln(pt))
  4: sum(d_fg*p)  5: sum(d_bg*p)     where W = pt-1 = 2pt'-p-t
"""

import numpy as np

H = 256
P = 128
HB = 2          # row halves: y = p + 128*h
WIN = 2         # y-window radius for stage 2
PAD = 8         # y-pad in transposed layout
BIG = 256.0     # "no pixel" sentinel (exact in bf16)
PADV = 65536.0  # pad value in the squared domain (= BIG**2, exact in bf16)
SEG = H + 2     # scan segment: [reset][256 cols][reset]
EPS = 1e-6
INF = 1e10
LN_BIAS = 1e-10
MAX_D2_OK = (WIN + 1) ** 2  # 9: windowed stage-2 exact iff its max <= this

_RUNNER = None


def _build_nc(loop_reps=None):
    import concourse.bacc as bacc
    import concourse.mybir as mybir
    import concourse.tile as tile

    dt = mybir.dt
    Alu = mybir.AluOpType
    Act = mybir.ActivationFunctionType

    from concourse import masks

    # Pin Exp/Ln/Square to the one ACT table that holds all three, so the
    # table-load pass never bounces between tables (each swap is ~1.3us).
    # Mutating the cached dict keeps table ids (json positions) intact.
    from concourse.hw_specs import get_activation_tables

    _tabs = get_activation_tables("gen3")
    _keep = {"natural_log_exp_and_others"}
    _funcs = {
        mybir.ActivationFunctionType.Exp,
        mybir.ActivationFunctionType.Ln,
        mybir.ActivationFunctionType.Square,
    }
    for _name, _s in _tabs.items():
        if _name not in _keep:
            for _f in _funcs:
                _s.discard(_f)

    nc = bacc.Bacc("TRN2", target_bir_lowering=False, debug=False, num_devices=8)
    # const AP for the Ln bias (only 0.0/1.0 are pre-registered)
    _bias_t = nc.alloc_sbuf_tensor("const-f32-lnbias", [P, 1], dt.float32)
    nc.gpsimd.memset(_bias_t.ap(), LN_BIAS)
    nc.const_aps.aps[(dt.float32, LN_BIAS)] = _bias_t.ap()
    nc.all_engine_barrier()
    pred = nc.dram_tensor("pred", [H, H], dt.float32, kind="ExternalInput")
    targ = nc.dram_tensor("targ", [H, H], dt.float32, kind="ExternalInput")
    stats_out = nc.dram_tensor("stats", [P, 8], dt.float32, kind="ExternalOutput")

    with tile.TileContext(nc) as tc:
        import contextlib

        SM = HB * SEG  # per-mask scan length

        with (
            tc.tile_pool(name="const", bufs=1) as cpool,
            tc.tile_pool(name="semi", bufs=1) as spool,
            tc.tile_pool(name="main", bufs=2) as pool,
            tc.tile_pool(name="psum", bufs=2, space="PSUM") as psum_pool,
        ):
            # ---- constants + semi-constant layouts (once, outside loop) ----
            ident = cpool.tile([P, P], dt.bfloat16)
            masks.make_identity(nc, ident[:])
            ONES = cpool.tile([P, SM], dt.bfloat16)
            Ovs = ONES[:].rearrange("p (h x) -> p h x", h=HB)
            nc.gpsimd.memset(ONES[:], 1.0)
            nc.gpsimd.memset(Ovs[:, :, 0:1], BIG)
            nc.gpsimd.memset(Ovs[:, :, SEG - 1 : SEG], BIG)

            # G holds BIG-masked indicators in the segmented scan layout;
            # sentinel columns are static, data columns rewritten per iter.
            G = spool.tile([P, 2, SM], dt.bfloat16)
            for m in range(2):
                Gmv = G[:, m].rearrange("p (h x) -> p h x", h=HB)
                nc.gpsimd.memset(Gmv[:, :, 0:1], BIG)
                nc.gpsimd.memset(Gmv[:, :, SEG - 1 : SEG], BIG)
            # S1T: squared row distances, x-major, with static y-pads
            S1T = spool.tile([P, 2, HB, H + 2 * PAD], dt.bfloat16)
            nc.gpsimd.memset(S1T[:, :, :, 0:PAD], PADV)
            nc.gpsimd.memset(S1T[:, :, :, PAD + H :], PADV)
            # stats lives across loop iterations; DMA'd out once after the
            # loop (keeps the SP queue free for next-iteration input DMAs)
            stats = spool.tile([P, 8], dt.float32)

            with tc.For_i(0, loop_reps, 1) if loop_reps else contextlib.nullcontext():
                # ---- DMA inputs (SP queue; targ first, it gates the EDT) ----
                tin = pool.tile([P, HB, H], dt.float32)
                tv = targ.ap().rearrange("(h p) x -> p h x", h=HB)
                nc.sync.dma_start(tin[:], tv[:])
                xin = pool.tile([P, HB, H], dt.float32)
                nc.scalar.dma_start(xin[:], pred.ap().rearrange("(h p) x -> p h x", h=HB))

                # ---- DVE: indicator builds + segmented scans (fg first) ----
                F = pool.tile([P, 2, SM], dt.bfloat16)
                M = pool.tile([P, 2, SM], dt.bfloat16)
                scan_ends = []
                for m in range(2):
                    Gmv = G[:, m].rearrange("p (h x) -> p h x", h=HB)
                    nc.vector.tensor_scalar(
                        Gmv[:, :, 1 : 1 + H], tin[:], 0.5, BIG,
                        op0=(Alu.is_le if m == 0 else Alu.is_gt), op1=Alu.mult,
                    )
                    nc.vector.tensor_tensor_scan(
                        F[:, m], ONES[:], G[:, m], BIG, op0=Alu.add, op1=Alu.min
                    )
                    sc = nc.vector.tensor_tensor_scan(
                        M[:, m, ::-1], ONES[:, ::-1], F[:, m, ::-1], BIG,
                        op0=Alu.add, op1=Alu.min,
                    )
                    scan_ends.append(sc)

                # ---- PE: block-transpose row distances into PSUM ----
                PS1a = psum_pool.tile([P, HB, H], dt.bfloat16, tag="ps1a")
                PS1b = psum_pool.tile([P, HB, H], dt.bfloat16, tag="ps1b")
                PS1 = [PS1a, PS1b]
                for m in range(2):
                    Mmv = M[:, m].rearrange("p (h x) -> p h x", h=HB)
                    for g in range(HB):
                        for h in range(HB):
                            nc.tensor.transpose(
                                PS1[m][:, g, P * h : P * h + P],
                                Mmv[:, h, 1 + P * g : 1 + P * g + P],
                                ident[:],
                            )

                # ---- ACT op 1: e = exp(-x); delayed so the in-order ACT
                # queue stays compact (span limits the loop period)
                E = pool.tile([P, HB, H], dt.bfloat16)
                nc.scalar.activation(E[:], xin[:], Act.Exp, scale=-1.0)
                # p = exp(-ln(1+e)) -- keeps everything in one ACT table
                LNU = pool.tile([P, HB, H], dt.float32)
                nc.scalar.activation(LNU[:], E[:], Act.Ln, bias=1.0)
                Pt = pool.tile([P, HB, H], dt.bfloat16)
                nc.scalar.activation(Pt[:], LNU[:], Act.Exp, scale=-1.0)

                # ---- ACT: square-evacuate PSUM -> padded S1T (s1 = r^2) ----
                for m in range(2):
                    nc.scalar.activation(
                        S1T[:, m, :, PAD : PAD + H], PS1[m][:], Act.Square
                    )
                # bf16 copy of t for the bf16 product chain
                TB = pool.tile([P, HB, H], dt.bfloat16)
                nc.vector.tensor_scalar(TB[:], tin[:], 1.0, None, op0=Alu.mult)

                # ---- SP: DMA-transpose p into x-major layout (bf16) ----
                PTF = pool.tile([P, HB, HB, P], dt.bfloat16, tag="ptf")
                for h in range(HB):
                    nc.sync.dma_start_transpose(PTF[:, h], Pt[:, h, :])

                # ---- DVE: dice/focal chain (bf16 TT/TTR) ----
                A2 = pool.tile([P, HB, H], dt.bfloat16)
                # col0 = 2*sum(p*t); A2 tensor = 2*p*t
                nc.vector.scalar_tensor_tensor(
                    A2[:], Pt[:], 2.0, TB[:], op0=Alu.mult, op1=Alu.mult,
                    accum_out=stats[:, 0:1],
                )
                V = pool.tile([P, HB, H], dt.bfloat16)
                # col1 = sum(p+t)
                nc.vector.scalar_tensor_tensor(
                    V[:], Pt[:], 1.0, TB[:], op0=Alu.mult, op1=Alu.add,
                    accum_out=stats[:, 1:2],
                )
                W = pool.tile([P, HB, H], dt.bfloat16)
                nc.vector.tensor_tensor(W[:], A2[:], V[:], op=Alu.subtract)
                SQ = pool.tile([P, HB, H], dt.bfloat16)
                nc.vector.tensor_tensor(SQ[:], W[:], W[:], op=Alu.mult)
                LNPT = pool.tile([P, HB, H], dt.bfloat16)
                nc.scalar.activation(LNPT[:], W[:], Act.Ln, bias=1.0)

                # ---- DVE: stage-2 window (shifted mins + offset tree) ----
                D2T = pool.tile([P, 2, HB, H], dt.bfloat16)
                LND = pool.tile([P, 2, HB, H], dt.float32)
                SD = pool.tile([P, 2, HB, H], dt.float32)
                for m in range(2):
                    C = S1T[:, m, :, PAD : PAD + H]
                    T1 = pool.tile([P, HB, H], dt.bfloat16, tag=f"t1{m}")
                    nc.vector.tensor_tensor(
                        T1[:],
                        S1T[:, m, :, PAD - 1 : PAD - 1 + H],
                        S1T[:, m, :, PAD + 1 : PAD + 1 + H],
                        op=Alu.min,
                    )
                    T2 = pool.tile([P, HB, H], dt.bfloat16, tag=f"t2{m}")
                    nc.vector.tensor_tensor(
                        T2[:],
                        S1T[:, m, :, PAD - 2 : PAD - 2 + H],
                        S1T[:, m, :, PAD + 2 : PAD + 2 + H],
                        op=Alu.min,
                    )
                    T1p = pool.tile([P, HB, H], dt.bfloat16, tag=f"t1p{m}")
                    nc.vector.tensor_scalar(T1p[:], T1[:], 1.0, None, op0=Alu.add)
                    A1 = pool.tile([P, HB, H], dt.bfloat16, tag=f"a1{m}")
                    nc.vector.tensor_tensor(A1[:], T1p[:], C, op=Alu.min)
                    T2p = pool.tile([P, HB, H], dt.bfloat16, tag=f"t2p{m}")
                    nc.vector.tensor_scalar(T2p[:], T2[:], 4.0, None, op0=Alu.add)
                    nc.vector.tensor_tensor(D2T[:, m], T2p[:], A1[:], op=Alu.min)
                    # boundary: d = exp(0.5*ln(d2)) on ACT, per mask
                    nc.scalar.activation(LND[:, m], D2T[:, m], Act.Ln, bias=LN_BIAS)
                    nc.scalar.activation(SD[:, m], LND[:, m], Act.Exp, scale=0.5)

                # focal accums
                F1 = pool.tile([P, HB, H], dt.bfloat16)
                # col2 = sum(W^2 * ln(pt))
                nc.vector.scalar_tensor_tensor(
                    F1[:], SQ[:], 1.0, LNPT[:], op0=Alu.mult, op1=Alu.mult,
                    accum_out=stats[:, 2:3],
                )
                F2 = pool.tile([P, HB, H], dt.bfloat16)
                # col3 = sum(t * W^2 * ln(pt))
                nc.vector.scalar_tensor_tensor(
                    F2[:], F1[:], 1.0, TB[:], op0=Alu.mult, op1=Alu.mult,
                    accum_out=stats[:, 3:4],
                )

                # boundary accums: col 4+m = sum(d_m * p). SD[:,m] is
                # [x, (g, h, y')]; reorder PTF [x, h, g, y'] once to match,
                # then both TTR operands are flat 2D (ISA AP limit).
                PTC = pool.tile([P, HB, HB, P], dt.bfloat16, tag="ptc")
                nc.vector.tensor_scalar(
                    PTC[:].rearrange("p g h y -> p h g y"), PTF[:], 1.0, None,
                    op0=Alu.mult,
                )
                for m in range(2):
                    FINm = pool.tile([P, HB, HB, P], dt.bfloat16, tag=f"fin{m}")
                    nc.vector.scalar_tensor_tensor(
                        FINm[:], SD[:, m].rearrange("p g (h y) -> p g h y", h=HB),
                        1.0, PTC[:], op0=Alu.mult, op1=Alu.mult,
                        accum_out=stats[:, 4 + m : 5 + m],
                    )

            nc.sync.dma_start(stats_out.ap()[:, 0:6], stats[:, 0:6])

    nc.compile()
    return nc


def _get_runner(loop_reps=None):
    """Build the Bass program + jitted PJRT executable once; return a
    callable (pred8, targ8) -> stats [8, 128, 8]."""
    global _RUNNER
    if _RUNNER is None:
        _RUNNER = {}
    if loop_reps in _RUNNER:
        return _RUNNER[loop_reps]

    import jax
    import concourse.mybir as mybir
    from concourse import bass2jax
    from jax.sharding import Mesh, PartitionSpec
    from jax.experimental.shard_map import shard_map

    bass2jax.install_neuronx_cc_hook()
    nc = _build_nc(loop_reps)

    n_cores = 8
    partition_name = (
        nc.partition_id_tensor.name if nc.partition_id_tensor else None
    )
    in_names, out_names, out_avals, zero_outs = [], [], [], []
    for alloc in nc.m.functions[0].allocations:
        if not isinstance(alloc, mybir.MemoryLocationSet):
            continue
        name = alloc.memorylocations[0].name
        if alloc.kind == "ExternalInput":
            if name != partition_name:
                in_names.append(name)
        elif alloc.kind == "ExternalOutput":
            shape = tuple(alloc.tensor_shape)
            dtype = mybir.dt.np(alloc.dtype)
            out_names.append(name)
            out_avals.append(jax.core.ShapedArray(shape, dtype))
            zero_outs.append(np.zeros(shape, dtype))
    n_params = len(in_names)
    all_names = in_names + out_names
    if partition_name is not None:
        all_names.append(partition_name)

    def _body(*args):
        operands = list(args)
        if partition_name is not None:
            operands.append(bass2jax.partition_id_tensor())
        outs = bass2jax._bass_exec_p.bind(
            *operands,
            out_avals=tuple(out_avals),
            in_names=tuple(all_names),
            out_names=tuple(out_names),
            lowering_input_output_aliases=(),
            sim_require_finite=True,
            sim_require_nnan=True,
            nc=nc,
        )
        return tuple(outs)

    devices = jax.devices()[:n_cores]
    mesh = Mesh(np.asarray(devices), ("core",))
    n_ops = n_params + len(out_names)
    sharded = jax.jit(
        shard_map(
            _body,
            mesh=mesh,
            in_specs=(PartitionSpec("core"),) * n_ops,
            out_specs=(PartitionSpec("core"),) * len(out_names),
            check_rep=False,
        ),
        donate_argnums=tuple(range(n_params, n_ops)),
        keep_unused=True,
    )
    concat_zero_shapes = [
        ((n_cores * z.shape[0],) + z.shape[1:], z.dtype) for z in zero_outs
    ]

    def run(pred8, targ8):
        ins = {"pred": pred8, "targ": targ8}
        concat_in = [
            np.ascontiguousarray(ins[name]).reshape(n_cores * H, H)
            for name in in_names
        ]
        zeros = [np.zeros(s, d) for s, d in concat_zero_shapes]
        out_arrs = sharded(*concat_in, *zeros)
        st = np.asarray(out_arrs[0])
        return st.reshape(n_cores, P, 8)

    _RUNNER[loop_reps] = run
    return run


# ---------------- host-side exact fallback (near-never path) ----------------

def _np_row_dist(mask):
    """Per-row 1D L1 distance to nearest True, BIG if row empty. [H,W]"""
    Hh, Wd = mask.shape
    f = np.full((Hh,), BIG, np.float32)
    out_f = np.empty((Hh, Wd), np.float32)
    for x in range(Wd):
        f = np.minimum(f + 1.0, np.where(mask[:, x], 0.0, BIG))
        out_f[:, x] = f
    b = np.full((Hh,), BIG, np.float32)
    out_b = np.empty((Hh, Wd), np.float32)
    for x in range(Wd - 1, -1, -1):
        b = np.minimum(b + 1.0, np.where(mask[:, x], 0.0, BIG))
        out_b[:, x] = b
    return np.minimum(out_f, out_b)


def _np_win_d2(mask):
    """Windowed stage-2 result (same algorithm as the device kernel)."""
    s1 = _np_row_dist(mask) ** 2
    Hh = s1.shape[0]
    pad = np.full((WIN, s1.shape[1]), PADV, np.float32)
    s1p = np.concatenate([pad, s1, pad], axis=0)
    d2 = s1.copy()
    for d in range(1, WIN + 1):
        m = np.minimum(s1p[WIN - d : WIN - d + Hh], s1p[WIN + d : WIN + d + Hh])
        d2 = np.minimum(d2, m + d * d)
    return d2


def _np_exact_edt(mask):
    """Exact EDT matching the reference formula (incl. empty-mask fallback)."""
    Hh, Wd = mask.shape
    ax = np.arange(Wd, dtype=np.float32)
    dx2 = (ax[:, None] - ax[None, :]) ** 2
    d1 = np.where(mask[:, None, :], dx2[None, :, :], INF).min(-1)
    ay = np.arange(Hh, dtype=np.float32)
    dy2 = (ay[:, None] - ay[None, :]) ** 2
    d = (dy2[:, :, None] + d1[None, :, :]).min(1)
    max_d2 = float((Hh - 1) ** 2 + (Wd - 1) ** 2)
    d = np.where(d > INF * 0.5, max_d2, d)
    return np.sqrt(d)


def _np_boundary_sum(pred_img, targ_img):
    """Exact sum(phi * sigmoid(pred)) for one image, reference semantics."""
    fg = targ_img > 0.5
    phi = np.where(fg, -_np_exact_edt(~fg), _np_exact_edt(fg))
    p = 1.0 / (1.0 + np.exp(-pred_img.astype(np.float64)))
    return float((phi.astype(np.float64) * p).sum())


# ---------------------------------- entry ----------------------------------

def kernel(pred_masks, target_masks):
    pred8 = np.asarray(pred_masks, dtype=np.float32).reshape(8, H, H)
    targ8 = np.asarray(target_masks, dtype=np.float32).reshape(8, H, H)

    stats = _get_runner()(pred8, targ8)  # [8, 128, 8]
    cols = stats.astype(np.float64).sum(axis=1)  # [8, 8]
    inter = cols[:, 0] / 2.0
    union = cols[:, 1]
    fsum = -0.75 * cols[:, 2] + 0.5 * cols[:, 3]
    bsum = cols[:, 4] - cols[:, 5]  # sum(d_fg*p) - sum(d_bg*p)

    n_el = float(H * H)

    # guard: windowed stage-2 certified exact iff its max <= (WIN+1)^2
    for i in range(8):
        fg = targ8[i] > 0.5
        if (not fg.any()) or fg.all() or \
           _np_win_d2(fg).max() > MAX_D2_OK or \
           _np_win_d2(~fg).max() > MAX_D2_OK:
            bsum[i] = _np_boundary_sum(pred8[i], targ8[i])

    ratios = (2.0 * inter + EPS) / (union + EPS)
    dice_val = 1.0 - ratios.mean()
    boundary_val = bsum.sum() / (8.0 * n_el)
    focal_val = fsum.sum() / (8.0 * n_el)
    loss = dice_val + boundary_val + focal_val
    return (
        np.float32(loss),
        np.float32(dice_val),
        np.float32(boundary_val),
        np.float32(focal_val),
    )


# revision 21
# speedup vs baseline: 1.0607x; 1.0607x over previous
"""Binary segmentation loss (dice + boundary + focal) on 8 Trainium2 cores.

Data parallel: image i -> core i. Each core computes six partial sums over
its image; the host combines them into the 4 scalar outputs.

Design (vs the v1 baseline kernel, ~2x faster):
- Single ACT function table (natural_log_exp_and_others), pinned via the
  cached table dict: sigmoid is p = exp(-ln(1+exp(-x))) and sqrt(d2) is
  exp(0.5*ln(d2)), so the 4x ~1.3us LoadActFuncSet swaps per iteration
  become one.
- EDT stage 1: exact separable row distance via fwd/bwd min-scans (DVE),
  PE block-transpose to x-major, ACT Square evacuation from PSUM.
- EDT stage 2: +-2 y-window min over r^2 + dy^2 in bf16 on DVE
  (shifted-min TTs + offset TS + min TT; no STT where TS+TT is cheaper).
- The boundary term never transposes d2 back: SD = exp(0.5*ln(d2)) is
  multiplied by a DMA-transposed (SBUF->SBUF xbar) copy of p and
  accumulated in the x-major layout (sums are layout-invariant).
- The stats tile lives across loop iterations (bufs=1) and is DMA'd once
  after the loop, keeping the SP queue free for input DMAs; tin goes on
  the SP queue, xin on the ACT queue so both are in flight at once.
- The EDT y-window is WIN=2. Exactness certificate: if the windowed d2 is
  <= (WIN+1)^2 = 9 everywhere, no |dy|>WIN candidate (>= 9) can beat it,
  so the windowed result equals the exact EDT. A host-side guard verifies
  this per mask and falls back to an exact numpy EDT for any image where
  it fails (P ~ 1e-2 per run for ~50% random masks; never on seed-0 data).

Engine notes learned on real TRN2 (kept for future work): GPSIMD/Pool
rejects min/STT ops and runs 2-input ops at ~2.6 cyc/elem, so everything
hot lives on DVE/ACT/PE; tensor_tensor_reduce and pow/divide ALU ops
compile but fault or fail ISA checks at runtime -- scalar_tensor_tensor
(+f32 accum_out) is the only HW-proven accumulating op.

Stats columns (per partition, host sums over partitions):
  0: 2*sum(p*t)   1: sum(p+t)   2: sum(W^2*ln(pt))   3: sum(t*W^2*ln(pt))
  4: sum(d_fg*p)  5: sum(d_bg*p)     where W = pt-1 = 2*p*t-p-t
"""

import numpy as np

H = 256
P = 128
HB = 2          # row halves: y = p + 128*h
WIN = 2         # y-window radius for stage 2
PAD = 8         # y-pad in transposed layout
BIG = 256.0     # "no pixel" sentinel (exact in bf16)
PADV = 65536.0  # pad value in the squared domain (= BIG**2, exact in bf16)
SEG = H + 2     # scan segment: [reset][256 cols][reset]
EPS = 1e-6
INF = 1e10
LN_BIAS = 1e-10
MAX_D2_OK = (WIN + 1) ** 2  # 9: windowed stage-2 exact iff its max <= this

_RUNNER = None


def _build_nc(loop_reps=None):
    import concourse.bacc as bacc
    import concourse.mybir as mybir
    import concourse.tile as tile

    dt = mybir.dt
    Alu = mybir.AluOpType
    Act = mybir.ActivationFunctionType

    from concourse import masks

    # Pin Exp/Ln/Square to the one ACT table that holds all three, so the
    # table-load pass never bounces between tables (each swap is ~1.3us).
    # Mutating the cached dict keeps table ids (json positions) intact.
    from concourse.hw_specs import get_activation_tables

    _tabs = get_activation_tables("gen3")
    _keep = {"natural_log_exp_and_others"}
    _funcs = {
        mybir.ActivationFunctionType.Exp,
        mybir.ActivationFunctionType.Ln,
        mybir.ActivationFunctionType.Square,
    }
    for _name, _s in _tabs.items():
        if _name not in _keep:
            for _f in _funcs:
                _s.discard(_f)

    nc = bacc.Bacc("TRN2", target_bir_lowering=False, debug=False, num_devices=8)
    # const AP for the Ln bias (only 0.0/1.0 are pre-registered)
    _bias_t = nc.alloc_sbuf_tensor("const-f32-lnbias", [P, 1], dt.float32)
    nc.gpsimd.memset(_bias_t.ap(), LN_BIAS)
    nc.const_aps.aps[(dt.float32, LN_BIAS)] = _bias_t.ap()
    nc.all_engine_barrier()
    pred = nc.dram_tensor("pred", [H, H], dt.float32, kind="ExternalInput")
    targ = nc.dram_tensor("targ", [H, H], dt.float32, kind="ExternalInput")
    stats_out = nc.dram_tensor("stats", [P, 8], dt.float32, kind="ExternalOutput")

    with tile.TileContext(nc) as tc:
        import contextlib

        SM = HB * SEG  # per-mask scan length

        with (
            tc.tile_pool(name="const", bufs=1) as cpool,
            tc.tile_pool(name="semi", bufs=1) as spool,
            tc.tile_pool(name="main", bufs=2) as pool,
            tc.tile_pool(name="psum", bufs=2, space="PSUM") as psum_pool,
        ):
            # ---- constants + semi-constant layouts (once, outside loop) ----
            ident = cpool.tile([P, P], dt.bfloat16)
            masks.make_identity(nc, ident[:])
            ONES = cpool.tile([P, SM], dt.bfloat16)
            Ovs = ONES[:].rearrange("p (h x) -> p h x", h=HB)
            nc.gpsimd.memset(ONES[:], 1.0)
            nc.gpsimd.memset(Ovs[:, :, 0:1], BIG)
            nc.gpsimd.memset(Ovs[:, :, SEG - 1 : SEG], BIG)

            # G holds BIG-masked indicators in the segmented scan layout;
            # sentinel columns are static, data columns rewritten per iter.
            G = spool.tile([P, 2, SM], dt.bfloat16)
            for m in range(2):
                Gmv = G[:, m].rearrange("p (h x) -> p h x", h=HB)
                nc.gpsimd.memset(Gmv[:, :, 0:1], BIG)
                nc.gpsimd.memset(Gmv[:, :, SEG - 1 : SEG], BIG)
            # S1T: squared row distances, x-major, with static y-pads
            S1T = spool.tile([P, 2, HB, H + 2 * PAD], dt.bfloat16)
            nc.gpsimd.memset(S1T[:, :, :, 0:PAD], PADV)
            nc.gpsimd.memset(S1T[:, :, :, PAD + H :], PADV)
            # stats lives across loop iterations; DMA'd out once after the
            # loop (keeps the SP queue free for next-iteration input DMAs)
            stats = spool.tile([P, 8], dt.float32)

            with tc.For_i(0, loop_reps, 1) if loop_reps else contextlib.nullcontext():
                # ---- DMA inputs (SP queue; targ first, it gates the EDT) ----
                tin = pool.tile([P, HB, H], dt.float32)
                tv = targ.ap().rearrange("(h p) x -> p h x", h=HB)
                nc.sync.dma_start(tin[:], tv[:])
                xin = pool.tile([P, HB, H], dt.float32)
                nc.scalar.dma_start(xin[:], pred.ap().rearrange("(h p) x -> p h x", h=HB))

                # ---- DVE: indicator builds + segmented scans (fg first) ----
                F = pool.tile([P, 2, SM], dt.bfloat16)
                M = pool.tile([P, 2, SM], dt.bfloat16)
                scan_ends = []
                for m in range(2):
                    Gmv = G[:, m].rearrange("p (h x) -> p h x", h=HB)
                    nc.vector.tensor_scalar(
                        Gmv[:, :, 1 : 1 + H], tin[:], 0.5, BIG,
                        op0=(Alu.is_le if m == 0 else Alu.is_gt), op1=Alu.mult,
                    )
                    nc.vector.tensor_tensor_scan(
                        F[:, m], ONES[:], G[:, m], BIG, op0=Alu.add, op1=Alu.min
                    )
                    sc = nc.vector.tensor_tensor_scan(
                        M[:, m, ::-1], ONES[:, ::-1], F[:, m, ::-1], BIG,
                        op0=Alu.add, op1=Alu.min,
                    )
                    scan_ends.append(sc)

                # ---- PE: block-transpose row distances into PSUM ----
                PS1a = psum_pool.tile([P, HB, H], dt.bfloat16, tag="ps1a")
                PS1b = psum_pool.tile([P, HB, H], dt.bfloat16, tag="ps1b")
                PS1 = [PS1a, PS1b]
                for m in range(2):
                    Mmv = M[:, m].rearrange("p (h x) -> p h x", h=HB)
                    for g in range(HB):
                        for h in range(HB):
                            nc.tensor.transpose(
                                PS1[m][:, g, P * h : P * h + P],
                                Mmv[:, h, 1 + P * g : 1 + P * g + P],
                                ident[:],
                            )

                # ---- ACT op 1: e = exp(-x); delayed so the in-order ACT
                # queue stays compact (span limits the loop period)
                E = pool.tile([P, HB, H], dt.bfloat16)
                nc.scalar.activation(E[:], xin[:], Act.Exp, scale=-1.0)
                # p = exp(-ln(1+e)) -- keeps everything in one ACT table
                LNU = pool.tile([P, HB, H], dt.float32)
                nc.scalar.activation(LNU[:], E[:], Act.Ln, bias=1.0)
                Pt = pool.tile([P, HB, H], dt.bfloat16)
                nc.scalar.activation(Pt[:], LNU[:], Act.Exp, scale=-1.0)

                # ---- ACT: square-evacuate PSUM -> padded S1T (s1 = r^2) ----
                for m in range(2):
                    nc.scalar.activation(
                        S1T[:, m, :, PAD : PAD + H], PS1[m][:], Act.Square
                    )
                # bf16 copy of t for the bf16 product chain
                TB = pool.tile([P, HB, H], dt.bfloat16)
                nc.vector.tensor_scalar(TB[:], tin[:], 1.0, None, op0=Alu.mult)

                # ---- SP: DMA-transpose p into x-major layout (bf16) ----
                PTF = pool.tile([P, HB, HB, P], dt.bfloat16, tag="ptf")
                for h in range(HB):
                    nc.sync.dma_start_transpose(PTF[:, h], Pt[:, h, :])

                # ---- DVE: dice/focal chain (bf16 TT/TTR) ----
                A2 = pool.tile([P, HB, H], dt.bfloat16)
                # col0 = 2*sum(p*t); A2 tensor = 2*p*t
                nc.vector.scalar_tensor_tensor(
                    A2[:], Pt[:], 2.0, TB[:], op0=Alu.mult, op1=Alu.mult,
                    accum_out=stats[:, 0:1],
                )
                V = pool.tile([P, HB, H], dt.bfloat16)
                # col1 = sum(p+t)
                nc.vector.scalar_tensor_tensor(
                    V[:], Pt[:], 1.0, TB[:], op0=Alu.mult, op1=Alu.add,
                    accum_out=stats[:, 1:2],
                )
                W = pool.tile([P, HB, H], dt.bfloat16)
                nc.vector.tensor_tensor(W[:], A2[:], V[:], op=Alu.subtract)
                SQ = pool.tile([P, HB, H], dt.bfloat16)
                nc.vector.tensor_tensor(SQ[:], W[:], W[:], op=Alu.mult)
                LNPT = pool.tile([P, HB, H], dt.bfloat16)
                nc.scalar.activation(LNPT[:], W[:], Act.Ln, bias=1.0)

                # ---- DVE: stage-2 window (shifted mins + offset tree) ----
                D2T = pool.tile([P, 2, HB, H], dt.bfloat16)
                LND = pool.tile([P, 2, HB, H], dt.float32)
                SD = pool.tile([P, 2, HB, H], dt.float32)
                for m in range(2):
                    C = S1T[:, m, :, PAD : PAD + H]
                    T1 = pool.tile([P, HB, H], dt.bfloat16, tag=f"t1{m}")
                    nc.vector.tensor_tensor(
                        T1[:],
                        S1T[:, m, :, PAD - 1 : PAD - 1 + H],
                        S1T[:, m, :, PAD + 1 : PAD + 1 + H],
                        op=Alu.min,
                    )
                    T2 = pool.tile([P, HB, H], dt.bfloat16, tag=f"t2{m}")
                    nc.vector.tensor_tensor(
                        T2[:],
                        S1T[:, m, :, PAD - 2 : PAD - 2 + H],
                        S1T[:, m, :, PAD + 2 : PAD + 2 + H],
                        op=Alu.min,
                    )
                    T1p = pool.tile([P, HB, H], dt.bfloat16, tag=f"t1p{m}")
                    nc.vector.tensor_scalar(T1p[:], T1[:], 1.0, None, op0=Alu.add)
                    A1 = pool.tile([P, HB, H], dt.bfloat16, tag=f"a1{m}")
                    nc.vector.tensor_tensor(A1[:], T1p[:], C, op=Alu.min)
                    T2p = pool.tile([P, HB, H], dt.bfloat16, tag=f"t2p{m}")
                    nc.vector.tensor_scalar(T2p[:], T2[:], 4.0, None, op0=Alu.add)
                    nc.vector.tensor_tensor(D2T[:, m], T2p[:], A1[:], op=Alu.min)
                    # boundary: d = exp(0.5*ln(d2)) on ACT, per mask
                    nc.scalar.activation(LND[:, m], D2T[:, m], Act.Ln, bias=LN_BIAS)
                    nc.scalar.activation(SD[:, m], LND[:, m], Act.Exp, scale=0.5)

                # focal accums
                F1 = pool.tile([P, HB, H], dt.bfloat16)
                # col2 = sum(W^2 * ln(pt))
                nc.vector.scalar_tensor_tensor(
                    F1[:], SQ[:], 1.0, LNPT[:], op0=Alu.mult, op1=Alu.mult,
                    accum_out=stats[:, 2:3],
                )
                F2 = pool.tile([P, HB, H], dt.bfloat16)
                # col3 = sum(t * W^2 * ln(pt))
                nc.vector.scalar_tensor_tensor(
                    F2[:], F1[:], 1.0, TB[:], op0=Alu.mult, op1=Alu.mult,
                    accum_out=stats[:, 3:4],
                )

                # boundary accums: col 4+m = sum(d_m * p). SD[:,m] is
                # [x, (g, h, y')]; reorder PTF [x, h, g, y'] once to match,
                # then both TTR operands are flat 2D (ISA AP limit).
                PTC = pool.tile([P, HB, HB, P], dt.bfloat16, tag="ptc")
                nc.vector.tensor_scalar(
                    PTC[:].rearrange("p g h y -> p h g y"), PTF[:], 1.0, None,
                    op0=Alu.mult,
                )
                for m in range(2):
                    FINm = pool.tile([P, HB, HB, P], dt.bfloat16, tag=f"fin{m}")
                    nc.vector.scalar_tensor_tensor(
                        FINm[:], SD[:, m].rearrange("p g (h y) -> p g h y", h=HB),
                        1.0, PTC[:], op0=Alu.mult, op1=Alu.mult,
                        accum_out=stats[:, 4 + m : 5 + m],
                    )

            nc.sync.dma_start(stats_out.ap()[:, 0:6], stats[:, 0:6])

    nc.compile()
    return nc


def _get_runner(loop_reps=None):
    """Build the Bass program + jitted PJRT executable once; return a
    callable (pred8, targ8) -> stats [8, 128, 8]."""
    global _RUNNER
    if _RUNNER is None:
        _RUNNER = {}
    if loop_reps in _RUNNER:
        return _RUNNER[loop_reps]

    import jax
    import concourse.mybir as mybir
    from concourse import bass2jax
    from jax.sharding import Mesh, PartitionSpec
    from jax.experimental.shard_map import shard_map

    bass2jax.install_neuronx_cc_hook()
    nc = _build_nc(loop_reps)

    n_cores = 8
    partition_name = (
        nc.partition_id_tensor.name if nc.partition_id_tensor else None
    )
    in_names, out_names, out_avals, zero_outs = [], [], [], []
    for alloc in nc.m.functions[0].allocations:
        if not isinstance(alloc, mybir.MemoryLocationSet):
            continue
        name = alloc.memorylocations[0].name
        if alloc.kind == "ExternalInput":
            if name != partition_name:
                in_names.append(name)
        elif alloc.kind == "ExternalOutput":
            shape = tuple(alloc.tensor_shape)
            dtype = mybir.dt.np(alloc.dtype)
            out_names.append(name)
            out_avals.append(jax.core.ShapedArray(shape, dtype))
            zero_outs.append(np.zeros(shape, dtype))
    n_params = len(in_names)
    all_names = in_names + out_names
    if partition_name is not None:
        all_names.append(partition_name)

    def _body(*args):
        operands = list(args)
        if partition_name is not None:
            operands.append(bass2jax.partition_id_tensor())
        outs = bass2jax._bass_exec_p.bind(
            *operands,
            out_avals=tuple(out_avals),
            in_names=tuple(all_names),
            out_names=tuple(out_names),
            lowering_input_output_aliases=(),
            sim_require_finite=True,
            sim_require_nnan=True,
            nc=nc,
        )
        return tuple(outs)

    devices = jax.devices()[:n_cores]
    mesh = Mesh(np.asarray(devices), ("core",))
    n_ops = n_params + len(out_names)
    sharded = jax.jit(
        shard_map(
            _body,
            mesh=mesh,
            in_specs=(PartitionSpec("core"),) * n_ops,
            out_specs=(PartitionSpec("core"),) * len(out_names),
            check_rep=False,
        ),
        donate_argnums=tuple(range(n_params, n_ops)),
        keep_unused=True,
    )
    concat_zero_shapes = [
        ((n_cores * z.shape[0],) + z.shape[1:], z.dtype) for z in zero_outs
    ]

    def run(pred8, targ8):
        ins = {"pred": pred8, "targ": targ8}
        concat_in = [
            np.ascontiguousarray(ins[name]).reshape(n_cores * H, H)
            for name in in_names
        ]
        zeros = [np.zeros(s, d) for s, d in concat_zero_shapes]
        out_arrs = sharded(*concat_in, *zeros)
        st = np.asarray(out_arrs[0])
        return st.reshape(n_cores, P, 8)

    _RUNNER[loop_reps] = run
    return run


# ---------------- host-side exact fallback (near-never path) ----------------

def _np_row_dist(mask):
    """Per-row 1D L1 distance to nearest True, BIG if row empty. [H,W]"""
    Hh, Wd = mask.shape
    f = np.full((Hh,), BIG, np.float32)
    out_f = np.empty((Hh, Wd), np.float32)
    for x in range(Wd):
        f = np.minimum(f + 1.0, np.where(mask[:, x], 0.0, BIG))
        out_f[:, x] = f
    b = np.full((Hh,), BIG, np.float32)
    out_b = np.empty((Hh, Wd), np.float32)
    for x in range(Wd - 1, -1, -1):
        b = np.minimum(b + 1.0, np.where(mask[:, x], 0.0, BIG))
        out_b[:, x] = b
    return np.minimum(out_f, out_b)


def _np_win_d2(mask):
    """Windowed stage-2 result (same algorithm as the device kernel)."""
    s1 = _np_row_dist(mask) ** 2
    Hh = s1.shape[0]
    pad = np.full((WIN, s1.shape[1]), PADV, np.float32)
    s1p = np.concatenate([pad, s1, pad], axis=0)
    d2 = s1.copy()
    for d in range(1, WIN + 1):
        m = np.minimum(s1p[WIN - d : WIN - d + Hh], s1p[WIN + d : WIN + d + Hh])
        d2 = np.minimum(d2, m + d * d)
    return d2


def _np_exact_edt(mask):
    """Exact EDT matching the reference formula (incl. empty-mask fallback)."""
    Hh, Wd = mask.shape
    ax = np.arange(Wd, dtype=np.float32)
    dx2 = (ax[:, None] - ax[None, :]) ** 2
    d1 = np.where(mask[:, None, :], dx2[None, :, :], INF).min(-1)
    ay = np.arange(Hh, dtype=np.float32)
    dy2 = (ay[:, None] - ay[None, :]) ** 2
    d = (dy2[:, :, None] + d1[None, :, :]).min(1)
    max_d2 = float((Hh - 1) ** 2 + (Wd - 1) ** 2)
    d = np.where(d > INF * 0.5, max_d2, d)
    return np.sqrt(d)


def _np_boundary_sum(pred_img, targ_img):
    """Exact sum(phi * sigmoid(pred)) for one image, reference semantics."""
    fg = targ_img > 0.5
    phi = np.where(fg, -_np_exact_edt(~fg), _np_exact_edt(fg))
    p = 1.0 / (1.0 + np.exp(-pred_img.astype(np.float64)))
    return float((phi.astype(np.float64) * p).sum())


# ---------------------------------- entry ----------------------------------

def kernel(pred_masks, target_masks):
    pred8 = np.asarray(pred_masks, dtype=np.float32).reshape(8, H, H)
    targ8 = np.asarray(target_masks, dtype=np.float32).reshape(8, H, H)

    stats = _get_runner()(pred8, targ8)  # [8, 128, 8]
    cols = stats.astype(np.float64).sum(axis=1)  # [8, 8]
    inter = cols[:, 0] / 2.0
    union = cols[:, 1]
    fsum = -0.75 * cols[:, 2] + 0.5 * cols[:, 3]
    bsum = cols[:, 4] - cols[:, 5]  # sum(d_fg*p) - sum(d_bg*p)

    n_el = float(H * H)

    # guard: windowed stage-2 certified exact iff its max <= (WIN+1)^2
    for i in range(8):
        fg = targ8[i] > 0.5
        if (not fg.any()) or fg.all() or \
           _np_win_d2(fg).max() > MAX_D2_OK or \
           _np_win_d2(~fg).max() > MAX_D2_OK:
            bsum[i] = _np_boundary_sum(pred8[i], targ8[i])

    ratios = (2.0 * inter + EPS) / (union + EPS)
    dice_val = 1.0 - ratios.mean()
    boundary_val = bsum.sum() / (8.0 * n_el)
    focal_val = fsum.sum() / (8.0 * n_el)
    loss = dice_val + boundary_val + focal_val
    return (
        np.float32(loss),
        np.float32(dice_val),
        np.float32(boundary_val),
        np.float32(focal_val),
    )


# revision 26
# speedup vs baseline: 1.2995x; 1.2252x over previous
"""Binary segmentation loss (dice + boundary + focal) on 8 Trainium2 cores.

Data parallel: image i -> core i. Each core computes six partial sums over
its image; the host combines them into the 4 scalar outputs.

Design (vs the v1 baseline kernel, ~2x faster):
- Single ACT function table (natural_log_exp_and_others), pinned via the
  cached table dict: sigmoid is p = exp(-ln(1+exp(-x))) and sqrt(d2) is
  exp(0.5*ln(d2)), so the 4x ~1.3us LoadActFuncSet swaps per iteration
  become one.
- EDT stage 1: exact separable row distance via fwd/bwd min-scans (DVE),
  PE block-transpose to x-major, ACT Square evacuation from PSUM.
- EDT stage 2: +-2 y-window min over r^2 + dy^2 in bf16 on DVE
  (shifted-min TTs + offset TS + min TT; no STT where TS+TT is cheaper).
- The boundary term never transposes d2 back: SD = exp(0.5*ln(d2)) is
  multiplied by a DMA-transposed (SBUF->SBUF xbar) copy of p and
  accumulated in the x-major layout (sums are layout-invariant).
- The stats tile lives across loop iterations (bufs=1) and is DMA'd once
  after the loop, keeping the SP queue free for input DMAs; tin goes on
  the SP queue, xin on the ACT queue so both are in flight at once.
- The EDT y-window is WIN=2. Exactness certificate: if the windowed d2 is
  <= (WIN+1)^2 = 9 everywhere, no |dy|>WIN candidate (>= 9) can beat it,
  so the windowed result equals the exact EDT. A host-side guard verifies
  this per mask and falls back to an exact numpy EDT for any image where
  it fails (P ~ 1e-2 per run for ~50% random masks; never on seed-0 data).

Engine notes learned on real TRN2 (kept for future work): GPSIMD/Pool
rejects min/STT ops and runs 2-input ops at ~2.6 cyc/elem, so everything
hot lives on DVE/ACT/PE; tensor_tensor_reduce and pow/divide ALU ops
compile but fault or fail ISA checks at runtime -- scalar_tensor_tensor
(+f32 accum_out) is the only HW-proven accumulating op.

Stats columns (per partition, host sums over partitions):
  0: 2*sum(p*t)   1: sum(p+t)   2: sum(W^2*ln(pt))   3: sum(t*W^2*ln(pt))
  4: sum(d_fg*p)  5: sum(d_bg*p)     where W = pt-1 = 2*p*t-p-t
"""

import numpy as np

H = 256
P = 128
HB = 2          # row halves: y = p + 128*h
WIN = 2         # y-window radius for stage 2
PAD = 8         # y-pad in transposed layout
BIG = 256.0     # "no pixel" sentinel (exact in bf16)
PADV = 65536.0  # pad value in the squared domain (= BIG**2, exact in bf16)
SEG = H + 2     # scan segment: [reset][256 cols][reset]
EPS = 1e-6
INF = 1e10
LN_BIAS = 1e-10
UNROLL = 2     # computations per hardware-loop iteration (amortizes the
               # loop barrier + ACT table load; scheduler overlaps the two)
MAX_D2_OK = (WIN + 1) ** 2  # 9: windowed stage-2 exact iff its max <= this

_RUNNER = None


def _build_nc(loop_reps=None):
    import concourse.bacc as bacc
    import concourse.mybir as mybir
    import concourse.tile as tile

    dt = mybir.dt
    Alu = mybir.AluOpType
    Act = mybir.ActivationFunctionType

    from concourse import masks

    # Pin Exp/Ln/Square to the one ACT table that holds all three, so the
    # table-load pass never bounces between tables (each swap is ~1.3us).
    # Mutating the cached dict keeps table ids (json positions) intact.
    from concourse.hw_specs import get_activation_tables

    _tabs = get_activation_tables("gen3")
    _keep = {"natural_log_exp_and_others"}
    _funcs = {
        mybir.ActivationFunctionType.Exp,
        mybir.ActivationFunctionType.Ln,
        mybir.ActivationFunctionType.Square,
    }
    for _name, _s in _tabs.items():
        if _name not in _keep:
            for _f in _funcs:
                _s.discard(_f)

    nc = bacc.Bacc("TRN2", target_bir_lowering=False, debug=False, num_devices=8)
    # const AP for the Ln bias (only 0.0/1.0 are pre-registered)
    _bias_t = nc.alloc_sbuf_tensor("const-f32-lnbias", [P, 1], dt.float32)
    nc.gpsimd.memset(_bias_t.ap(), LN_BIAS)
    nc.const_aps.aps[(dt.float32, LN_BIAS)] = _bias_t.ap()
    nc.all_engine_barrier()
    pred = nc.dram_tensor("pred", [H, H], dt.float32, kind="ExternalInput")
    targ = nc.dram_tensor("targ", [H, H], dt.float32, kind="ExternalInput")
    stats_out = nc.dram_tensor("stats", [P, 16], dt.float32, kind="ExternalOutput")

    with tile.TileContext(nc) as tc:
        import contextlib

        SM = HB * SEG  # per-mask scan length

        with (
            tc.tile_pool(name="const", bufs=1) as cpool,
            tc.tile_pool(name="semi", bufs=1) as spool,
            tc.tile_pool(name="main", bufs=1) as pool,
            tc.tile_pool(name="psum", bufs=1, space="PSUM") as psum_pool,
        ):
            # ---- constants + semi-constant layouts (once, outside loop) ----
            ident = cpool.tile([P, P], dt.bfloat16)
            masks.make_identity(nc, ident[:])
            ONES = cpool.tile([P, SM], dt.bfloat16)
            Ovs = ONES[:].rearrange("p (h x) -> p h x", h=HB)
            nc.gpsimd.memset(ONES[:], 1.0)
            nc.gpsimd.memset(Ovs[:, :, 0:1], BIG)
            nc.gpsimd.memset(Ovs[:, :, SEG - 1 : SEG], BIG)

            # G holds BIG-masked indicators in the segmented scan layout;
            # sentinel columns are static, data columns rewritten per iter.
            # One G/S1T per unroll slot so the two computations don't collide.
            Gs, S1Ts = [], []
            for u in range(UNROLL):
                G = spool.tile([P, 2, SM], dt.bfloat16, tag=f"g{u}")
                for m in range(2):
                    Gmv = G[:, m].rearrange("p (h x) -> p h x", h=HB)
                    nc.gpsimd.memset(Gmv[:, :, 0:1], BIG)
                    nc.gpsimd.memset(Gmv[:, :, SEG - 1 : SEG], BIG)
                S1T = spool.tile(
                    [P, 2, HB, H + 2 * PAD], dt.bfloat16, tag=f"s1t{u}"
                )
                nc.gpsimd.memset(S1T[:, :, :, 0:PAD], PADV)
                nc.gpsimd.memset(S1T[:, :, :, PAD + H :], PADV)
                Gs.append(G)
                S1Ts.append(S1T)
            # stats lives across loop iterations; DMA'd out once after the
            # loop (keeps the SP queue free for input DMAs). Cols 8u+k.
            stats = spool.tile([P, 16], dt.float32)
            nc.gpsimd.memset(stats[:], 0.0)

            def emit_iter(u):
                G = Gs[u]
                S1T = S1Ts[u]
                sc = 8 * u  # stats column base for this slot

                # ---- DMA inputs (tin on SP, xin on ACT queue) ----
                tin = pool.tile([P, HB, H], dt.float32, tag=f"tin{u}")
                tv = targ.ap().rearrange("(h p) x -> p h x", h=HB)
                nc.sync.dma_start(tin[:], tv[:])
                xin = pool.tile([P, HB, H], dt.float32, tag=f"xin{u}")
                nc.sync.dma_start(
                    xin[:], pred.ap().rearrange("(h p) x -> p h x", h=HB)
                )

                # ---- DVE: indicator builds + segmented scans (fg first) ----
                F = pool.tile([P, 2, SM], dt.bfloat16, tag=f"f{u}")
                M = pool.tile([P, 2, SM], dt.bfloat16, tag=f"m{u}")
                G0v = G[:, 0].rearrange("p (h x) -> p h x", h=HB)
                G1v = G[:, 1].rearrange("p (h x) -> p h x", h=HB)
                nc.vector.tensor_scalar(
                    G0v[:, :, 1 : 1 + H], tin[:], 0.5, BIG,
                    op0=Alu.is_le, op1=Alu.mult,
                )
                nc.vector.tensor_scalar(
                    G1v[:, :, 1 : 1 + H], G0v[:, :, 1 : 1 + H], -1.0, BIG,
                    op0=Alu.mult, op1=Alu.add,
                )
                for m in range(2):
                    nc.vector.tensor_tensor_scan(
                        F[:, m], ONES[:], G[:, m], BIG, op0=Alu.add, op1=Alu.min
                    )
                    nc.vector.tensor_tensor_scan(
                        M[:, m, ::-1], ONES[:, ::-1], F[:, m, ::-1], BIG,
                        op0=Alu.add, op1=Alu.min,
                    )

                # ---- PE: block-transpose row distances into PSUM ----
                PS1a = psum_pool.tile([P, HB, H], dt.bfloat16, tag=f"ps1a{u}")
                PS1b = psum_pool.tile([P, HB, H], dt.bfloat16, tag=f"ps1b{u}")
                PS1 = [PS1a, PS1b]
                for m in range(2):
                    Mmv = M[:, m].rearrange("p (h x) -> p h x", h=HB)
                    for g in range(HB):
                        for h in range(HB):
                            nc.tensor.transpose(
                                PS1[m][:, g, P * h : P * h + P],
                                Mmv[:, h, 1 + P * g : 1 + P * g + P],
                                ident[:],
                            )

                # ---- ACT: p = exp(-ln(1+exp(-x))), one table all kernel ----
                E = pool.tile([P, HB, H], dt.bfloat16, tag=f"e{u}")
                nc.scalar.activation(E[:], xin[:], Act.Exp, scale=-1.0)
                LNU = pool.tile([P, HB, H], dt.float32, tag=f"lnu{u}")
                nc.scalar.activation(LNU[:], E[:], Act.Ln, bias=1.0)
                Pt = pool.tile([P, HB, H], dt.bfloat16, tag=f"pt{u}")
                nc.scalar.activation(Pt[:], LNU[:], Act.Exp, scale=-1.0)

                # ---- ACT: square-evacuate PSUM -> padded S1T (s1 = r^2) ----
                for m in range(2):
                    nc.scalar.activation(
                        S1T[:, m, :, PAD : PAD + H], PS1[m][:], Act.Square
                    )
                # ---- SP: DMA-transpose p into x-major layout (bf16) ----
                PTF = pool.tile([P, HB, HB, P], dt.bfloat16, tag=f"ptf{u}")
                for h in range(HB):
                    nc.sync.dma_start_transpose(PTF[:, h], Pt[:, h, :])

                # ---- DVE: dice/focal chain (bf16) ----
                A2 = pool.tile([P, HB, H], dt.bfloat16, tag=f"a2{u}")
                # col0 = 2*sum(p*t); A2 tensor = 2*p*t
                nc.vector.scalar_tensor_tensor(
                    A2[:], Pt[:], 2.0, tin[:], op0=Alu.mult, op1=Alu.mult,
                    accum_out=stats[:, sc + 0 : sc + 1],
                )
                V = pool.tile([P, HB, H], dt.bfloat16, tag=f"v{u}")
                # col1 = sum(p+t)
                nc.vector.scalar_tensor_tensor(
                    V[:], Pt[:], 1.0, tin[:], op0=Alu.mult, op1=Alu.add,
                    accum_out=stats[:, sc + 1 : sc + 2],
                )
                W = pool.tile([P, HB, H], dt.bfloat16, tag=f"w{u}")
                nc.vector.tensor_tensor(W[:], A2[:], V[:], op=Alu.subtract)
                SQ = pool.tile([P, HB, H], dt.bfloat16, tag=f"sq{u}")
                nc.vector.tensor_tensor(SQ[:], W[:], W[:], op=Alu.mult)
                LNPT = pool.tile([P, HB, H], dt.bfloat16, tag=f"lnpt{u}")
                nc.scalar.activation(LNPT[:], W[:], Act.Ln, bias=1.0)

                # ---- DVE: stage-2 window + ACT sqrt, per mask ----
                D2T = pool.tile([P, 2, HB, H], dt.bfloat16, tag=f"d2t{u}")
                LND = pool.tile([P, 2, HB, H], dt.float32, tag=f"lnd{u}")
                SD = pool.tile([P, 2, HB, H], dt.float32, tag=f"sd{u}")
                for m in range(2):
                    C = S1T[:, m, :, PAD : PAD + H]
                    T1 = pool.tile([P, HB, H], dt.bfloat16, tag=f"t1{m}{u}")
                    nc.vector.tensor_tensor(
                        T1[:],
                        S1T[:, m, :, PAD - 1 : PAD - 1 + H],
                        S1T[:, m, :, PAD + 1 : PAD + 1 + H],
                        op=Alu.min,
                    )
                    T2 = pool.tile([P, HB, H], dt.bfloat16, tag=f"t2{m}{u}")
                    nc.vector.tensor_tensor(
                        T2[:],
                        S1T[:, m, :, PAD - 2 : PAD - 2 + H],
                        S1T[:, m, :, PAD + 2 : PAD + 2 + H],
                        op=Alu.min,
                    )
                    T1p = pool.tile([P, HB, H], dt.bfloat16, tag=f"t1p{m}{u}")
                    nc.scalar.activation(T1p[:], T1[:], Act.Copy, bias=1.0)
                    A1 = pool.tile([P, HB, H], dt.bfloat16, tag=f"a1{m}{u}")
                    nc.vector.tensor_tensor(A1[:], T1p[:], C, op=Alu.min)
                    T2p = pool.tile([P, HB, H], dt.bfloat16, tag=f"t2p{m}{u}")
                    nc.scalar.activation(T2p[:], T2[:], Act.Copy, bias=4.0)
                    nc.vector.tensor_tensor(D2T[:, m], T2p[:], A1[:], op=Alu.min)
                    # boundary: d = exp(0.5*ln(d2)) on ACT, per mask
                    nc.scalar.activation(LND[:, m], D2T[:, m], Act.Ln, bias=LN_BIAS)
                    nc.scalar.activation(SD[:, m], LND[:, m], Act.Exp, scale=0.5)

                # focal accums
                F1 = pool.tile([P, HB, H], dt.bfloat16, tag=f"f1{u}")
                # col2 = sum(W^2 * ln(pt))
                nc.vector.scalar_tensor_tensor(
                    F1[:], SQ[:], 1.0, LNPT[:], op0=Alu.mult, op1=Alu.mult,
                    accum_out=stats[:, sc + 2 : sc + 3],
                )
                F2 = pool.tile([P, HB, H], dt.bfloat16, tag=f"f2{u}")
                # col3 = sum(t * W^2 * ln(pt))
                nc.vector.scalar_tensor_tensor(
                    F2[:], F1[:], 1.0, tin[:], op0=Alu.mult, op1=Alu.mult,
                    accum_out=stats[:, sc + 3 : sc + 4],
                )

                # boundary accums: col 4+m = sum(d_m * p). STT inputs are
                # limited to 3D, so reorder PTF [x,h,g,y'] -> [x,g,h,y'] once.
                PTC = pool.tile([P, HB, HB, P], dt.bfloat16, tag=f"ptc{u}")
                nc.vector.tensor_scalar(
                    PTC[:].rearrange("p g h y -> p h g y"), PTF[:], 1.0, None,
                    op0=Alu.mult,
                )
                for m in range(2):
                    FINm = pool.tile(
                        [P, HB, HB, P], dt.bfloat16, tag=f"fin{m}{u}"
                    )
                    nc.vector.scalar_tensor_tensor(
                        FINm[:],
                        SD[:, m].rearrange("p g (h y) -> p g h y", h=HB),
                        1.0, PTC[:], op0=Alu.mult, op1=Alu.mult,
                        accum_out=stats[:, sc + 4 + m : sc + 5 + m],
                    )

            with tc.For_i(0, loop_reps, 1) if loop_reps else contextlib.nullcontext():
                for u in range(UNROLL):
                    emit_iter(u)

            nc.sync.dma_start(stats_out.ap()[:, 0:16], stats[:, 0:16])


    nc.compile()
    return nc


def _get_runner(loop_reps=None):
    """Build the Bass program + jitted PJRT executable once; return a
    callable (pred8, targ8) -> stats [8, 128, 8]."""
    global _RUNNER
    if _RUNNER is None:
        _RUNNER = {}
    if loop_reps in _RUNNER:
        return _RUNNER[loop_reps]

    import jax
    import concourse.mybir as mybir
    from concourse import bass2jax
    from jax.sharding import Mesh, PartitionSpec
    from jax.experimental.shard_map import shard_map

    bass2jax.install_neuronx_cc_hook()
    nc = _build_nc(loop_reps)

    n_cores = 8
    partition_name = (
        nc.partition_id_tensor.name if nc.partition_id_tensor else None
    )
    in_names, out_names, out_avals, zero_outs = [], [], [], []
    for alloc in nc.m.functions[0].allocations:
        if not isinstance(alloc, mybir.MemoryLocationSet):
            continue
        name = alloc.memorylocations[0].name
        if alloc.kind == "ExternalInput":
            if name != partition_name:
                in_names.append(name)
        elif alloc.kind == "ExternalOutput":
            shape = tuple(alloc.tensor_shape)
            dtype = mybir.dt.np(alloc.dtype)
            out_names.append(name)
            out_avals.append(jax.core.ShapedArray(shape, dtype))
            zero_outs.append(np.zeros(shape, dtype))
    n_params = len(in_names)
    all_names = in_names + out_names
    if partition_name is not None:
        all_names.append(partition_name)

    def _body(*args):
        operands = list(args)
        if partition_name is not None:
            operands.append(bass2jax.partition_id_tensor())
        outs = bass2jax._bass_exec_p.bind(
            *operands,
            out_avals=tuple(out_avals),
            in_names=tuple(all_names),
            out_names=tuple(out_names),
            lowering_input_output_aliases=(),
            sim_require_finite=True,
            sim_require_nnan=True,
            nc=nc,
        )
        return tuple(outs)

    devices = jax.devices()[:n_cores]
    mesh = Mesh(np.asarray(devices), ("core",))
    n_ops = n_params + len(out_names)
    sharded = jax.jit(
        shard_map(
            _body,
            mesh=mesh,
            in_specs=(PartitionSpec("core"),) * n_ops,
            out_specs=(PartitionSpec("core"),) * len(out_names),
            check_rep=False,
        ),
        donate_argnums=tuple(range(n_params, n_ops)),
        keep_unused=True,
    )
    concat_zero_shapes = [
        ((n_cores * z.shape[0],) + z.shape[1:], z.dtype) for z in zero_outs
    ]

    def run(pred8, targ8):
        ins = {"pred": pred8, "targ": targ8}
        concat_in = [
            np.ascontiguousarray(ins[name]).reshape(n_cores * H, H)
            for name in in_names
        ]
        zeros = [np.zeros(s, d) for s, d in concat_zero_shapes]
        out_arrs = sharded(*concat_in, *zeros)
        st = np.asarray(out_arrs[0])
        return st.reshape(n_cores, P, -1)

    _RUNNER[loop_reps] = run
    return run


# ---------------- host-side exact fallback (near-never path) ----------------

def _np_row_dist(mask):
    """Per-row 1D L1 distance to nearest True, BIG if row empty. [H,W]"""
    Hh, Wd = mask.shape
    f = np.full((Hh,), BIG, np.float32)
    out_f = np.empty((Hh, Wd), np.float32)
    for x in range(Wd):
        f = np.minimum(f + 1.0, np.where(mask[:, x], 0.0, BIG))
        out_f[:, x] = f
    b = np.full((Hh,), BIG, np.float32)
    out_b = np.empty((Hh, Wd), np.float32)
    for x in range(Wd - 1, -1, -1):
        b = np.minimum(b + 1.0, np.where(mask[:, x], 0.0, BIG))
        out_b[:, x] = b
    return np.minimum(out_f, out_b)


def _np_win_d2(mask):
    """Windowed stage-2 result (same algorithm as the device kernel)."""
    s1 = _np_row_dist(mask) ** 2
    Hh = s1.shape[0]
    pad = np.full((WIN, s1.shape[1]), PADV, np.float32)
    s1p = np.concatenate([pad, s1, pad], axis=0)
    d2 = s1.copy()
    for d in range(1, WIN + 1):
        m = np.minimum(s1p[WIN - d : WIN - d + Hh], s1p[WIN + d : WIN + d + Hh])
        d2 = np.minimum(d2, m + d * d)
    return d2


def _np_exact_edt(mask):
    """Exact EDT matching the reference formula (incl. empty-mask fallback)."""
    Hh, Wd = mask.shape
    ax = np.arange(Wd, dtype=np.float32)
    dx2 = (ax[:, None] - ax[None, :]) ** 2
    d1 = np.where(mask[:, None, :], dx2[None, :, :], INF).min(-1)
    ay = np.arange(Hh, dtype=np.float32)
    dy2 = (ay[:, None] - ay[None, :]) ** 2
    d = (dy2[:, :, None] + d1[None, :, :]).min(1)
    max_d2 = float((Hh - 1) ** 2 + (Wd - 1) ** 2)
    d = np.where(d > INF * 0.5, max_d2, d)
    return np.sqrt(d)


def _np_boundary_sum(pred_img, targ_img):
    """Exact sum(phi * sigmoid(pred)) for one image, reference semantics."""
    fg = targ_img > 0.5
    phi = np.where(fg, -_np_exact_edt(~fg), _np_exact_edt(fg))
    p = 1.0 / (1.0 + np.exp(-pred_img.astype(np.float64)))
    return float((phi.astype(np.float64) * p).sum())


# ---------------------------------- entry ----------------------------------

def kernel(pred_masks, target_masks):
    pred8 = np.asarray(pred_masks, dtype=np.float32).reshape(8, H, H)
    targ8 = np.asarray(target_masks, dtype=np.float32).reshape(8, H, H)

    stats = _get_runner()(pred8, targ8)  # [8, 128, 16]; cols 0:6 = slot u0
    cols = stats.astype(np.float64).sum(axis=1)  # [8, 8]
    inter = cols[:, 0] / 2.0
    union = cols[:, 1]
    fsum = -0.75 * cols[:, 2] + 0.5 * cols[:, 3]
    bsum = cols[:, 4] - cols[:, 5]  # sum(d_fg*p) - sum(d_bg*p)

    n_el = float(H * H)

    # guard: windowed stage-2 certified exact iff its max <= (WIN+1)^2
    for i in range(8):
        fg = targ8[i] > 0.5
        if (not fg.any()) or fg.all() or \
           _np_win_d2(fg).max() > MAX_D2_OK or \
           _np_win_d2(~fg).max() > MAX_D2_OK:
            bsum[i] = _np_boundary_sum(pred8[i], targ8[i])

    ratios = (2.0 * inter + EPS) / (union + EPS)
    dice_val = 1.0 - ratios.mean()
    boundary_val = bsum.sum() / (8.0 * n_el)
    focal_val = fsum.sum() / (8.0 * n_el)
    loss = dice_val + boundary_val + focal_val
    return (
        np.float32(loss),
        np.float32(dice_val),
        np.float32(boundary_val),
        np.float32(focal_val),
    )


# revision 28
# speedup vs baseline: 1.4900x; 1.1466x over previous
"""Binary segmentation loss (dice + boundary + focal) on 8 Trainium2 cores.

Data parallel: image i -> core i. Each core computes six partial sums over
its image; the host combines them into the 4 scalar outputs.

Design (vs the v1 baseline kernel, ~2x faster):
- Single ACT function table (natural_log_exp_and_others), pinned via the
  cached table dict: sigmoid is p = exp(-ln(1+exp(-x))) and sqrt(d2) is
  exp(0.5*ln(d2)), so the 4x ~1.3us LoadActFuncSet swaps per iteration
  become one.
- EDT stage 1: exact separable row distance via fwd/bwd min-scans (DVE),
  PE block-transpose to x-major, ACT Square evacuation from PSUM.
- EDT stage 2: +-2 y-window min over r^2 + dy^2 in bf16 on DVE
  (shifted-min TTs + offset TS + min TT; no STT where TS+TT is cheaper).
- The boundary term never transposes d2 back: SD = exp(0.5*ln(d2)) is
  multiplied by a DMA-transposed (SBUF->SBUF xbar) copy of p and
  accumulated in the x-major layout (sums are layout-invariant).
- The stats tile lives across loop iterations (bufs=1) and is DMA'd once
  after the loop, keeping the SP queue free for input DMAs; tin goes on
  the SP queue, xin on the ACT queue so both are in flight at once.
- The EDT y-window is WIN=2. Exactness certificate: if the windowed d2 is
  <= (WIN+1)^2 = 9 everywhere, no |dy|>WIN candidate (>= 9) can beat it,
  so the windowed result equals the exact EDT. A host-side guard verifies
  this per mask and falls back to an exact numpy EDT for any image where
  it fails (P ~ 1e-2 per run for ~50% random masks; never on seed-0 data).

Engine notes learned on real TRN2 (kept for future work): GPSIMD/Pool
rejects min/STT ops and runs 2-input ops at ~2.6 cyc/elem, so everything
hot lives on DVE/ACT/PE; tensor_tensor_reduce and pow/divide ALU ops
compile but fault or fail ISA checks at runtime -- scalar_tensor_tensor
(+f32 accum_out) is the only HW-proven accumulating op.

Stats columns (per partition, host sums over partitions):
  0: 2*sum(p*t)   1: sum(p+t)   2: sum(W^2*ln(pt))   3: sum(t*W^2*ln(pt))
  4: sum(d_fg*p)  5: sum(d_bg*p)     where W = pt-1 = 2*p*t-p-t
"""

import numpy as np

H = 256
P = 128
HB = 2          # row halves: y = p + 128*h
WIN = 2         # y-window radius for stage 2
PAD = 8         # y-pad in transposed layout
BIG = 256.0     # "no pixel" sentinel (exact in bf16)
PADV = 65536.0  # pad value in the squared domain (= BIG**2, exact in bf16)
SEG = H + 2     # scan segment: [reset][256 cols][reset]
EPS = 1e-6
INF = 1e10
LN_BIAS = 1e-10
UNROLL = 4     # computations per hardware-loop iteration (amortizes the
               # loop barrier + ACT table load; scheduler overlaps the two)
MAX_D2_OK = (WIN + 1) ** 2  # 9: windowed stage-2 exact iff its max <= this

_RUNNER = None


def _build_nc(loop_reps=None):
    import concourse.bacc as bacc
    import concourse.mybir as mybir
    import concourse.tile as tile

    dt = mybir.dt
    Alu = mybir.AluOpType
    Act = mybir.ActivationFunctionType

    from concourse import masks

    # Pin Exp/Ln/Square to the one ACT table that holds all three, so the
    # table-load pass never bounces between tables (each swap is ~1.3us).
    # Mutating the cached dict keeps table ids (json positions) intact.
    from concourse.hw_specs import get_activation_tables

    _tabs = get_activation_tables("gen3")
    _keep = {"natural_log_exp_and_others"}
    _funcs = {
        mybir.ActivationFunctionType.Exp,
        mybir.ActivationFunctionType.Ln,
        mybir.ActivationFunctionType.Square,
    }
    for _name, _s in _tabs.items():
        if _name not in _keep:
            for _f in _funcs:
                _s.discard(_f)

    nc = bacc.Bacc("TRN2", target_bir_lowering=False, debug=False, num_devices=8)
    # const AP for the Ln bias (only 0.0/1.0 are pre-registered)
    _bias_t = nc.alloc_sbuf_tensor("const-f32-lnbias", [P, 1], dt.float32)
    nc.gpsimd.memset(_bias_t.ap(), LN_BIAS)
    nc.const_aps.aps[(dt.float32, LN_BIAS)] = _bias_t.ap()
    nc.all_engine_barrier()
    pred = nc.dram_tensor("pred", [H, H], dt.float32, kind="ExternalInput")
    targ = nc.dram_tensor("targ", [H, H], dt.float32, kind="ExternalInput")
    stats_out = nc.dram_tensor("stats", [P, 8 * UNROLL], dt.float32, kind="ExternalOutput")

    with tile.TileContext(nc) as tc:
        import contextlib

        SM = HB * SEG  # per-mask scan length

        with (
            tc.tile_pool(name="const", bufs=1) as cpool,
            tc.tile_pool(name="semi", bufs=1) as spool,
            tc.tile_pool(name="main", bufs=1) as pool,
            tc.tile_pool(name="psum", bufs=1, space="PSUM") as psum_pool,
        ):
            # ---- constants + semi-constant layouts (once, outside loop) ----
            ident = cpool.tile([P, P], dt.bfloat16)
            masks.make_identity(nc, ident[:])
            ONES = cpool.tile([P, SM], dt.bfloat16)
            Ovs = ONES[:].rearrange("p (h x) -> p h x", h=HB)
            nc.gpsimd.memset(ONES[:], 1.0)
            nc.gpsimd.memset(Ovs[:, :, 0:1], BIG)
            nc.gpsimd.memset(Ovs[:, :, SEG - 1 : SEG], BIG)

            # G holds BIG-masked indicators in the segmented scan layout;
            # sentinel columns are static, data columns rewritten per iter.
            # One G/S1T per unroll slot so the two computations don't collide.
            Gs, S1Ts = [], []
            for u in range(UNROLL):
                G = spool.tile([P, 2, SM], dt.bfloat16, tag=f"g{u}")
                for m in range(2):
                    Gmv = G[:, m].rearrange("p (h x) -> p h x", h=HB)
                    nc.gpsimd.memset(Gmv[:, :, 0:1], BIG)
                    nc.gpsimd.memset(Gmv[:, :, SEG - 1 : SEG], BIG)
                S1T = spool.tile(
                    [P, 2, HB, H + 2 * PAD], dt.bfloat16, tag=f"s1t{u}"
                )
                nc.gpsimd.memset(S1T[:, :, :, 0:PAD], PADV)
                nc.gpsimd.memset(S1T[:, :, :, PAD + H :], PADV)
                Gs.append(G)
                S1Ts.append(S1T)
            # stats lives across loop iterations; DMA'd out once after the
            # loop (keeps the SP queue free for input DMAs). Cols 8u+k.
            stats = spool.tile([P, 8 * UNROLL], dt.float32)
            nc.gpsimd.memset(stats[:], 0.0)

            def emit_iter(u):
                G = Gs[u]
                S1T = S1Ts[u]
                sc = 8 * u  # stats column base for this slot

                # ---- DMA inputs (tin on SP, xin on ACT queue) ----
                tin = pool.tile([P, HB, H], dt.float32, tag=f"tin{u}")
                tv = targ.ap().rearrange("(h p) x -> p h x", h=HB)
                nc.sync.dma_start(tin[:], tv[:])
                xin = pool.tile([P, HB, H], dt.float32, tag=f"xin{u}")
                nc.sync.dma_start(
                    xin[:], pred.ap().rearrange("(h p) x -> p h x", h=HB)
                )

                # ---- DVE: indicator builds + segmented scans (fg first) ----
                F = pool.tile([P, 2, SM], dt.bfloat16, tag=f"f{u}")
                M = pool.tile([P, 2, SM], dt.bfloat16, tag=f"m{u}")
                G0v = G[:, 0].rearrange("p (h x) -> p h x", h=HB)
                G1v = G[:, 1].rearrange("p (h x) -> p h x", h=HB)
                nc.vector.tensor_scalar(
                    G0v[:, :, 1 : 1 + H], tin[:], 0.5, BIG,
                    op0=Alu.is_le, op1=Alu.mult,
                )
                nc.vector.tensor_scalar(
                    G1v[:, :, 1 : 1 + H], G0v[:, :, 1 : 1 + H], -1.0, BIG,
                    op0=Alu.mult, op1=Alu.add,
                )
                for m in range(2):
                    nc.vector.tensor_tensor_scan(
                        F[:, m], ONES[:], G[:, m], BIG, op0=Alu.add, op1=Alu.min
                    )
                    nc.vector.tensor_tensor_scan(
                        M[:, m, ::-1], ONES[:, ::-1], F[:, m, ::-1], BIG,
                        op0=Alu.add, op1=Alu.min,
                    )

                # ---- PE: block-transpose row distances into PSUM ----
                PS1a = psum_pool.tile([P, HB, H], dt.bfloat16, tag=f"ps1a{u}")
                PS1b = psum_pool.tile([P, HB, H], dt.bfloat16, tag=f"ps1b{u}")
                PS1 = [PS1a, PS1b]
                for m in range(2):
                    Mmv = M[:, m].rearrange("p (h x) -> p h x", h=HB)
                    for g in range(HB):
                        for h in range(HB):
                            nc.tensor.transpose(
                                PS1[m][:, g, P * h : P * h + P],
                                Mmv[:, h, 1 + P * g : 1 + P * g + P],
                                ident[:],
                            )

                # ---- ACT: p = exp(-ln(1+exp(-x))), one table all kernel ----
                E = pool.tile([P, HB, H], dt.bfloat16, tag=f"e{u}")
                nc.scalar.activation(E[:], xin[:], Act.Exp, scale=-1.0)
                LNU = pool.tile([P, HB, H], dt.float32, tag=f"lnu{u}")
                nc.scalar.activation(LNU[:], E[:], Act.Ln, bias=1.0)
                Pt = pool.tile([P, HB, H], dt.bfloat16, tag=f"pt{u}")
                nc.scalar.activation(Pt[:], LNU[:], Act.Exp, scale=-1.0)

                # ---- ACT: square-evacuate PSUM -> padded S1T (s1 = r^2) ----
                for m in range(2):
                    nc.scalar.activation(
                        S1T[:, m, :, PAD : PAD + H], PS1[m][:], Act.Square
                    )
                # ---- SP: DMA-transpose p into x-major layout (bf16) ----
                PTF = pool.tile([P, HB, HB, P], dt.bfloat16, tag=f"ptf{u}")
                for h in range(HB):
                    nc.sync.dma_start_transpose(PTF[:, h], Pt[:, h, :])

                # ---- DVE: dice/focal chain (bf16) ----
                A2 = pool.tile([P, HB, H], dt.bfloat16, tag=f"a2{u}")
                # col0 = 2*sum(p*t); A2 tensor = 2*p*t
                nc.vector.scalar_tensor_tensor(
                    A2[:], Pt[:], 2.0, tin[:], op0=Alu.mult, op1=Alu.mult,
                    accum_out=stats[:, sc + 0 : sc + 1],
                )
                V = pool.tile([P, HB, H], dt.bfloat16, tag=f"v{u}")
                # col1 = sum(p+t)
                nc.vector.scalar_tensor_tensor(
                    V[:], Pt[:], 1.0, tin[:], op0=Alu.mult, op1=Alu.add,
                    accum_out=stats[:, sc + 1 : sc + 2],
                )
                W = pool.tile([P, HB, H], dt.bfloat16, tag=f"w{u}")
                nc.vector.tensor_tensor(W[:], A2[:], V[:], op=Alu.subtract)
                SQ = pool.tile([P, HB, H], dt.bfloat16, tag=f"sq{u}")
                nc.vector.tensor_tensor(SQ[:], W[:], W[:], op=Alu.mult)
                LNPT = pool.tile([P, HB, H], dt.bfloat16, tag=f"lnpt{u}")
                nc.scalar.activation(LNPT[:], W[:], Act.Ln, bias=1.0)

                # ---- DVE: stage-2 window + ACT sqrt, per mask ----
                D2T = pool.tile([P, 2, HB, H], dt.bfloat16, tag=f"d2t{u}")
                LND = pool.tile([P, 2, HB, H], dt.float32, tag=f"lnd{u}")
                SD = pool.tile([P, 2, HB, H], dt.float32, tag=f"sd{u}")
                for m in range(2):
                    C = S1T[:, m, :, PAD : PAD + H]
                    T1 = pool.tile([P, HB, H], dt.bfloat16, tag=f"t1{m}{u}")
                    nc.vector.tensor_tensor(
                        T1[:],
                        S1T[:, m, :, PAD - 1 : PAD - 1 + H],
                        S1T[:, m, :, PAD + 1 : PAD + 1 + H],
                        op=Alu.min,
                    )
                    T2 = pool.tile([P, HB, H], dt.bfloat16, tag=f"t2{m}{u}")
                    nc.vector.tensor_tensor(
                        T2[:],
                        S1T[:, m, :, PAD - 2 : PAD - 2 + H],
                        S1T[:, m, :, PAD + 2 : PAD + 2 + H],
                        op=Alu.min,
                    )
                    T1p = pool.tile([P, HB, H], dt.bfloat16, tag=f"t1p{m}{u}")
                    nc.scalar.activation(T1p[:], T1[:], Act.Copy, bias=1.0)
                    A1 = pool.tile([P, HB, H], dt.bfloat16, tag=f"a1{m}{u}")
                    nc.vector.tensor_tensor(A1[:], T1p[:], C, op=Alu.min)
                    T2p = pool.tile([P, HB, H], dt.bfloat16, tag=f"t2p{m}{u}")
                    nc.scalar.activation(T2p[:], T2[:], Act.Copy, bias=4.0)
                    nc.vector.tensor_tensor(D2T[:, m], T2p[:], A1[:], op=Alu.min)
                    # boundary: d = exp(0.5*ln(d2)) on ACT, per mask
                    nc.scalar.activation(LND[:, m], D2T[:, m], Act.Ln, bias=LN_BIAS)
                    nc.scalar.activation(SD[:, m], LND[:, m], Act.Exp, scale=0.5)

                # focal accums
                F1 = pool.tile([P, HB, H], dt.bfloat16, tag=f"f1{u}")
                # col2 = sum(W^2 * ln(pt))
                nc.vector.scalar_tensor_tensor(
                    F1[:], SQ[:], 1.0, LNPT[:], op0=Alu.mult, op1=Alu.mult,
                    accum_out=stats[:, sc + 2 : sc + 3],
                )
                F2 = pool.tile([P, HB, H], dt.bfloat16, tag=f"f2{u}")
                # col3 = sum(t * W^2 * ln(pt))
                nc.vector.scalar_tensor_tensor(
                    F2[:], F1[:], 1.0, tin[:], op0=Alu.mult, op1=Alu.mult,
                    accum_out=stats[:, sc + 3 : sc + 4],
                )

                # boundary accums: col 4+m = sum(d_m * p). STT inputs are
                # limited to 3D, so reorder PTF [x,h,g,y'] -> [x,g,h,y'] once.
                PTC = pool.tile([P, HB, HB, P], dt.bfloat16, tag=f"ptc{u}")
                nc.vector.tensor_scalar(
                    PTC[:].rearrange("p g h y -> p h g y"), PTF[:], 1.0, None,
                    op0=Alu.mult,
                )
                for m in range(2):
                    FINm = pool.tile(
                        [P, HB, HB, P], dt.bfloat16, tag=f"fin{m}{u}"
                    )
                    nc.vector.scalar_tensor_tensor(
                        FINm[:],
                        SD[:, m].rearrange("p g (h y) -> p g h y", h=HB),
                        1.0, PTC[:], op0=Alu.mult, op1=Alu.mult,
                        accum_out=stats[:, sc + 4 + m : sc + 5 + m],
                    )

            with tc.For_i(0, loop_reps, 1) if loop_reps else contextlib.nullcontext():
                for u in range(UNROLL):
                    emit_iter(u)

            nc.sync.dma_start(stats_out.ap()[:], stats[:])


    nc.compile()
    return nc


def _get_runner(loop_reps=None):
    """Build the Bass program + jitted PJRT executable once; return a
    callable (pred8, targ8) -> stats [8, 128, 8]."""
    global _RUNNER
    if _RUNNER is None:
        _RUNNER = {}
    if loop_reps in _RUNNER:
        return _RUNNER[loop_reps]

    import jax
    import concourse.mybir as mybir
    from concourse import bass2jax
    from jax.sharding import Mesh, PartitionSpec
    from jax.experimental.shard_map import shard_map

    bass2jax.install_neuronx_cc_hook()
    nc = _build_nc(loop_reps)

    n_cores = 8
    partition_name = (
        nc.partition_id_tensor.name if nc.partition_id_tensor else None
    )
    in_names, out_names, out_avals, zero_outs = [], [], [], []
    for alloc in nc.m.functions[0].allocations:
        if not isinstance(alloc, mybir.MemoryLocationSet):
            continue
        name = alloc.memorylocations[0].name
        if alloc.kind == "ExternalInput":
            if name != partition_name:
                in_names.append(name)
        elif alloc.kind == "ExternalOutput":
            shape = tuple(alloc.tensor_shape)
            dtype = mybir.dt.np(alloc.dtype)
            out_names.append(name)
            out_avals.append(jax.core.ShapedArray(shape, dtype))
            zero_outs.append(np.zeros(shape, dtype))
    n_params = len(in_names)
    all_names = in_names + out_names
    if partition_name is not None:
        all_names.append(partition_name)

    def _body(*args):
        operands = list(args)
        if partition_name is not None:
            operands.append(bass2jax.partition_id_tensor())
        outs = bass2jax._bass_exec_p.bind(
            *operands,
            out_avals=tuple(out_avals),
            in_names=tuple(all_names),
            out_names=tuple(out_names),
            lowering_input_output_aliases=(),
            sim_require_finite=True,
            sim_require_nnan=True,
            nc=nc,
        )
        return tuple(outs)

    devices = jax.devices()[:n_cores]
    mesh = Mesh(np.asarray(devices), ("core",))
    n_ops = n_params + len(out_names)
    sharded = jax.jit(
        shard_map(
            _body,
            mesh=mesh,
            in_specs=(PartitionSpec("core"),) * n_ops,
            out_specs=(PartitionSpec("core"),) * len(out_names),
            check_rep=False,
        ),
        donate_argnums=tuple(range(n_params, n_ops)),
        keep_unused=True,
    )
    concat_zero_shapes = [
        ((n_cores * z.shape[0],) + z.shape[1:], z.dtype) for z in zero_outs
    ]

    def run(pred8, targ8):
        ins = {"pred": pred8, "targ": targ8}
        concat_in = [
            np.ascontiguousarray(ins[name]).reshape(n_cores * H, H)
            for name in in_names
        ]
        zeros = [np.zeros(s, d) for s, d in concat_zero_shapes]
        out_arrs = sharded(*concat_in, *zeros)
        st = np.asarray(out_arrs[0])
        return st.reshape(n_cores, P, -1)

    _RUNNER[loop_reps] = run
    return run


# ---------------- host-side exact fallback (near-never path) ----------------

def _np_row_dist(mask):
    """Per-row 1D L1 distance to nearest True, BIG if row empty. [H,W]"""
    Hh, Wd = mask.shape
    f = np.full((Hh,), BIG, np.float32)
    out_f = np.empty((Hh, Wd), np.float32)
    for x in range(Wd):
        f = np.minimum(f + 1.0, np.where(mask[:, x], 0.0, BIG))
        out_f[:, x] = f
    b = np.full((Hh,), BIG, np.float32)
    out_b = np.empty((Hh, Wd), np.float32)
    for x in range(Wd - 1, -1, -1):
        b = np.minimum(b + 1.0, np.where(mask[:, x], 0.0, BIG))
        out_b[:, x] = b
    return np.minimum(out_f, out_b)


def _np_win_d2(mask):
    """Windowed stage-2 result (same algorithm as the device kernel)."""
    s1 = _np_row_dist(mask) ** 2
    Hh = s1.shape[0]
    pad = np.full((WIN, s1.shape[1]), PADV, np.float32)
    s1p = np.concatenate([pad, s1, pad], axis=0)
    d2 = s1.copy()
    for d in range(1, WIN + 1):
        m = np.minimum(s1p[WIN - d : WIN - d + Hh], s1p[WIN + d : WIN + d + Hh])
        d2 = np.minimum(d2, m + d * d)
    return d2


def _np_exact_edt(mask):
    """Exact EDT matching the reference formula (incl. empty-mask fallback)."""
    Hh, Wd = mask.shape
    ax = np.arange(Wd, dtype=np.float32)
    dx2 = (ax[:, None] - ax[None, :]) ** 2
    d1 = np.where(mask[:, None, :], dx2[None, :, :], INF).min(-1)
    ay = np.arange(Hh, dtype=np.float32)
    dy2 = (ay[:, None] - ay[None, :]) ** 2
    d = (dy2[:, :, None] + d1[None, :, :]).min(1)
    max_d2 = float((Hh - 1) ** 2 + (Wd - 1) ** 2)
    d = np.where(d > INF * 0.5, max_d2, d)
    return np.sqrt(d)


def _np_boundary_sum(pred_img, targ_img):
    """Exact sum(phi * sigmoid(pred)) for one image, reference semantics."""
    fg = targ_img > 0.5
    phi = np.where(fg, -_np_exact_edt(~fg), _np_exact_edt(fg))
    p = 1.0 / (1.0 + np.exp(-pred_img.astype(np.float64)))
    return float((phi.astype(np.float64) * p).sum())


# ---------------------------------- entry ----------------------------------

def kernel(pred_masks, target_masks):
    pred8 = np.asarray(pred_masks, dtype=np.float32).reshape(8, H, H)
    targ8 = np.asarray(target_masks, dtype=np.float32).reshape(8, H, H)

    stats = _get_runner()(pred8, targ8)  # [8, 128, 16]; cols 0:6 = slot u0
    cols = stats.astype(np.float64).sum(axis=1)  # [8, 8]
    inter = cols[:, 0] / 2.0
    union = cols[:, 1]
    fsum = -0.75 * cols[:, 2] + 0.5 * cols[:, 3]
    bsum = cols[:, 4] - cols[:, 5]  # sum(d_fg*p) - sum(d_bg*p)

    n_el = float(H * H)

    # guard: windowed stage-2 certified exact iff its max <= (WIN+1)^2
    for i in range(8):
        fg = targ8[i] > 0.5
        if (not fg.any()) or fg.all() or \
           _np_win_d2(fg).max() > MAX_D2_OK or \
           _np_win_d2(~fg).max() > MAX_D2_OK:
            bsum[i] = _np_boundary_sum(pred8[i], targ8[i])

    ratios = (2.0 * inter + EPS) / (union + EPS)
    dice_val = 1.0 - ratios.mean()
    boundary_val = bsum.sum() / (8.0 * n_el)
    focal_val = fsum.sum() / (8.0 * n_el)
    loss = dice_val + boundary_val + focal_val
    return (
        np.float32(loss),
        np.float32(dice_val),
        np.float32(boundary_val),
        np.float32(focal_val),
    )
